# revision 41
# baseline (speedup 1.0000x reference)
"""LoRA layer kernel for Trainium2 (Bass/Tile), data-parallel over 8 NeuronCores.

Math:  out = (x @ B) @ A * (32/16)   with x [4,2048,4096], B [4096,16], A [16,4096].

Design (HBM-bound: ~8 MB in + ~8 MB out per core at f16; floor ~42-47 us):
  - Flatten tokens (4*2048=8192), shard 1024 tokens per core (data parallel).
  - FINE-GRAINED pipeline: 8 blocks of 128 tokens (1 MB in / 1 MB out each)
    so the first store dispatches ~8 us after the first load lands and the
    store stream is smooth (1 MB every ~2.7 us, alternating rings).
  - x pre-tiled PARTITION-MAJOR on host as [ntb, 128, NB, tb] f16; loads
    use >=8 KB-per-partition descriptors (smaller descriptors measured ~2x
    slower per byte on this HW). x0/x1 in halves (separate completion
    semaphores -> mm1 starts ~2 us earlier), x2.. whole-block, ALL
    dispatched up-front (x fully SBUF-buffered -> queue never starves).
  - Two HWDGE rings, loads and stores interleaving at SDMA packet granularity:
      q1  (nc.sync):   B, x0..x7, stores of blocks ntb/2..
      q10 (nc.scalar): A (compact), stores of blocks 0..ntb/2-1
    Early-block stores go on q10 (empty after the small A load); late-block
    stores on q1 (loads fully dispatched/drained by then).
  - A loaded compact [32, 4096] (rows 16..32 host-zeroed) and replicated to
    rows 32g..32g+32 with three 16-bit SBUF->SBUF copies (DVE 4x perf mode)
    while the copy engines are otherwise idle; avoids ~0.9 MB of HBM traffic
    in the critical early window.
  - mm1: 4-way column-group packed fp16 matmuls with B zero-padded to 32
    cols per group; chunk 4k+g accumulates into PSUM rows [32g, 32g+32)
    where rows 32g+16.. are exact zeros.
  - NO DVE fold: ONE [128, tb] PSUM->SBUF f16 copy moves all 4 partial
    groups (zeros included) into xbp; mm2 contracts K=128 against
    A_sb whose zero rows annihilate the padding => implicit fold, full
    PE-row utilization, no row-packing needed.
  - PSUM->SBUF output copies [128, 512] split by o-PARITY (even chunk ->
    DVE, odd -> ACT): both engines drain every block concurrently, halving
    per-block copy latency; the shared 6-buffer mm2 PSUM pool keeps the PE
    only loosely coupled to the copy stream.
  - 8 PE warm-up matmuls against B right after the B load keep the PE busy
    before x0 arrives so the DVFS p-state ramps early.

  Measured on HW: 60.7-66 us (vs 72.8 us baseline); run-to-run spread is
  dominated by chip-level util throttling (throttle_activity_1 = 0.5x
  limiter active 18-34% of a run, varies by run).
"""

import os
import numpy as np

IN = 4096
OUT = 4096
R = 16
N_CORES = 8
SCALE = 32.0 / 16.0
P = 128
NB = IN // P  # 32 contraction chunks


def _install_profile_hook():
    """Best-effort: register the axon NTFF profiling hook that this image's
    `antenv` package is missing, so run_bass_kernel_spmd(trace=True) can
    return exec_time_ns. Harmless no-op when anything is unavailable."""
    try:
        import sys
        import types

        if "antenv.axon_hooks" in sys.modules:
            return
        try:
            import antenv  # noqa: F401
        except ImportError:
            return
        mod = types.ModuleType("antenv.axon_hooks")
        mod._hook = None

        def set_axon_ntff_profile_hook(h):
            mod._hook = h

        def get_axon_ntff_profile_hook():
            return mod._hook

        mod.set_axon_ntff_profile_hook = set_axon_ntff_profile_hook
        mod.get_axon_ntff_profile_hook = get_axon_ntff_profile_hook
        sys.modules["antenv.axon_hooks"] = mod
        import antenv as _antenv

        _antenv.axon_hooks = mod

        so_path = "/opt/axon/libaxon_pjrt.so"
        if os.path.exists(so_path):
            try:
                from trn_agent_boot.trn_boot import _ntff_profile_via_ctypes

                hook = _ntff_profile_via_ctypes(so_path)
                if hook is not None:
                    mod._hook = hook
            except Exception:
                pass
    except Exception:
        pass


_install_profile_hook()

_NC_CACHE = {}


def build_nc(tok, tb=128):
    """Build + compile the per-core Bass program for `tok` tokens/core."""
    key = (tok, tb)
    if key in _NC_CACHE:
        return _NC_CACHE[key]

    import concourse.bacc as bacc
    import concourse.tile as tile
    from concourse import mybir

    f32 = mybir.dt.float32
    f16 = mybir.dt.float16
    tb = min(tb, tok)
    assert tok % tb == 0 and tb % P == 0
    ntb = tok // tb
    assert tb == P, "fine-grained pipeline assumes one subtile per block"

    nc = bacc.Bacc("TRN2", target_bir_lowering=False, debug=False)
    xT = nc.dram_tensor("xT", [ntb, P, NB, tb], f16, kind="ExternalInput").ap()
    Bt = nc.dram_tensor("Bt", [P, NB, 2 * R], f16, kind="ExternalInput").ap()
    Af = nc.dram_tensor("Af", [2 * R, OUT], f16, kind="ExternalInput").ap()
    out = nc.dram_tensor("out", [tok, OUT], f16, kind="ExternalOutput").ap()

    with tile.TileContext(nc) as tc:
        with (
            tc.tile_pool(name="const", bufs=1) as const_pool,
            tc.tile_pool(name="xin", bufs=ntb) as x_pool,
            tc.tile_pool(name="xbp", bufs=3) as xbp_pool,
            tc.tile_pool(name="ps1", bufs=2, space="PSUM") as ps1,
            tc.tile_pool(name="ps2", bufs=6, space="PSUM") as ps2,
            tc.tile_pool(name="osb", bufs=min(4, ntb)) as out_pool,
        ):
            xT_sbs = [
                x_pool.tile([P, NB, tb], f16, name=f"x{i}", tag="x")
                for i in range(ntb)
            ]
            # B FIRST at the head of the big sync queue (gates mm1
            # LDWEIGHTS + PE warm-up); A on the scalar ring in parallel;
            # q10 is then free for the early-block stores
            B_sb = const_pool.tile([P, NB, 2 * R], f16)
            nc.sync.dma_start(out=B_sb[:], in_=Bt[:])
            A_sb = const_pool.tile([P, OUT], f16)
            nc.scalar.dma_start(out=A_sb[0 : 2 * R, :], in_=Af[:])
            # x0/x1 in halves so mm1 starts ~2 us earlier (each DMA's
            # completion semaphore costs ~1-2 us of HBM-receipt latency);
            # later blocks whole-block fat DMAs, all dispatched up-front
            for tbi in range(ntb):
                if tbi < 2 and NB % 2 == 0:
                    nh = NB // 2
                    for q in range(2):
                        nc.sync.dma_start(
                            out=xT_sbs[tbi][:, q * nh : (q + 1) * nh, :],
                            in_=xT[tbi, :, q * nh : (q + 1) * nh, :],
                        )
                else:
                    nc.sync.dma_start(out=xT_sbs[tbi][:], in_=xT[tbi])
            # replicate A (rows 0..16 data, 16..32 zeros) to the other three
            # row groups with 16-bit SBUF->SBUF copies (DVE 4x perf mode);
            # engines are idle this early
            nc.vector.tensor_copy(A_sb[32:64, :], A_sb[0:32, :])
            nc.vector.tensor_copy(A_sb[64:96, :], A_sb[0:32, :])
            nc.scalar.activation(
                A_sb[96:128, :], A_sb[0:32, :], mybir.ActivationFunctionType.Copy
            )

            # PE warm-up: dependency-free matmuls on B data right after the
            # B load; keeps the PE clock ramping while x0 streams in.
            warm_ps = ps1.tile([P, tb], f32, name="warm", tag="ps1")
            for w in range(8):
                nc.tensor.matmul(
                    warm_ps[0 : 2 * R, :],
                    lhsT=B_sb[:, 0, :],
                    rhs=B_sb[:, (w % 4) * 4 : (w % 4) * 4 + tb // (2 * R), :],
                    start=True,
                    stop=True,
                    tile_position=(0, 0),
                    skip_group_check=True,
                )

            # mm1 for block `tbi`, group c8 only (4 column-group-packed
            # matmuls); group g writes rows [32g, 32g+32) with the top 16
            # rows exact zeros
            def emit_mm1_group(tbi, ps_part, c8):
                xT_sb = xT_sbs[tbi]
                for g in range(4):
                    c = c8 * 4 + g
                    nc.tensor.matmul(
                        ps_part[32 * g : 32 * g + 2 * R, :],
                        lhsT=B_sb[:, c, :],
                        rhs=xT_sb[:, c, :],
                        start=(c8 == 0),
                        stop=(c8 == NB // 4 - 1),
                        tile_position=(0, 32 * g),
                        skip_group_check=True,
                    )

            def emit_xbp(tbi, ps_part):
                # single full-width copy of all 4 partial groups (+ zeros)
                # into mm2's K=128 weight layout; alternate engine per block
                xbp_sb = xbp_pool.tile([P, tb], f16, name=f"xbp{tbi}", tag="xbp")
                if tbi % 2 == 0:
                    nc.vector.tensor_copy(xbp_sb[:], ps_part[:])
                else:
                    nc.scalar.activation(
                        xbp_sb[:], ps_part[:], mybir.ActivationFunctionType.Copy
                    )
                return xbp_sb

            def new_ps1(tbi):
                return ps1.tile([P, tb], f32, name=f"ps1_{tbi}", tag="ps1")

            def emit_front(tbi):
                ps_part = new_ps1(tbi)
                for c8 in range(NB // 4):
                    emit_mm1_group(tbi, ps_part, c8)
                return emit_xbp(tbi, ps_part)

            def emit_back(tbi, xbp_sb):
                o_sb = out_pool.tile(
                    [P, OUT], f16, name=f"osb_{tbi}", tag=f"osb{tbi % 2}"
                )
                for o in range(OUT // 512):
                    ps_o = ps2.tile([P, 512], f32, tag="ps2")
                    nc.tensor.matmul(
                        ps_o[:],
                        lhsT=xbp_sb[:],
                        rhs=A_sb[:, o * 512 : (o + 1) * 512],
                        start=True,
                        stop=True,
                        tile_position=(0, 0),
                        skip_group_check=True,
                    )
                    # o-parity copy split: BOTH engines drain every block
                    # concurrently — halves per-block copy latency (incl.
                    # the final block's tail with no successor to overlap)
                    dst = o_sb[:, o * 512 : (o + 1) * 512]
                    if o % 2 == 0:
                        nc.vector.tensor_copy(dst, ps_o[:])
                    else:
                        nc.scalar.activation(
                            dst, ps_o[:], mybir.ActivationFunctionType.Copy
                        )
                t0 = tbi * tb
                if tbi < ntb // 2 or ntb == 1:
                    # early blocks: q10 is free right after the small A load
                    nc.scalar.dma_start(out=out[t0 : t0 + P, :], in_=o_sb[:])
                else:
                    # late blocks: sync's loads have drained by now
                    nc.sync.dma_start(out=out[t0 : t0 + P, :], in_=o_sb[:])

            # Software-pipelined by one block: PE order is
            # mm1(b), mm1(b+1), mm2(b), mm1(b+2), mm2(b+1) ...
            prev = None
            for tbi in range(ntb):
                xbp = emit_front(tbi)
                if prev is not None:
                    emit_back(prev[0], prev[1])
                prev = (tbi, xbp)
            emit_back(prev[0], prev[1])

    nc.compile()
    _NC_CACHE[key] = nc
    return nc


TB = 128


def make_in_maps(x, lora_A, lora_B, n_cores=N_CORES):
    x = np.asarray(x, dtype=np.float32)
    A = np.asarray(lora_A, dtype=np.float32)
    B = np.asarray(lora_B, dtype=np.float32)
    xf = x.reshape(-1, IN)
    ntok = xf.shape[0] // n_cores
    tb = min(TB, ntok)
    A_scaled = (A * np.float32(SCALE)).astype(np.float16)
    # Af: rows 0..16 hold A*scale, rows 16..32 EXACT zeros; replicated
    # on-chip to rows 32g.. so the zero rows annihilate the zero-padded
    # partial rows of mm1 (implicit fold in mm2's K=128 contraction)
    Af = np.zeros((2 * R, OUT), dtype=np.float16)
    Af[:R] = A_scaled
    B_resh = np.zeros((P, NB, 2 * R), dtype=np.float16)
    B_resh[:, :, :R] = B.reshape(NB, P, R).transpose(1, 0, 2)
    in_maps = []
    for c in range(n_cores):
        shard = xf[c * ntok : (c + 1) * ntok]
        # pre-tile partition-major: [ntb, 128, NB, tb];
        # xt[tbi, p, c, t] = shard[tbi*tb + t, c*128 + p]
        xt = np.ascontiguousarray(
            shard.reshape(ntok // tb, tb, NB, P).transpose(0, 3, 2, 1),
            dtype=np.float16,
        )
        in_maps.append(
            {
                "xT": xt,
                "Bt": B_resh,
                "Af": Af,
            }
        )
    return in_maps, ntok


def kernel_with_results(x, lora_A, lora_B, trace=False, **kwargs):
    from concourse.bass_utils import run_bass_kernel_spmd

    in_maps, ntok = make_in_maps(x, lora_A, lora_B)
    nc = build_nc(ntok, tb=TB)
    res = run_bass_kernel_spmd(nc, in_maps, list(range(N_CORES)), trace=trace, **kwargs)
    out = np.concatenate([r["out"] for r in res.results], axis=0).astype(np.float32)
    return out.reshape(np.asarray(x).shape[:-1] + (OUT,)), res


def kernel(x, lora_A, lora_B):
    out, _ = kernel_with_results(x, lora_A, lora_B)
    return out
